# revision 10
# baseline (speedup 1.0000x reference)
"""BERT self-attention (B=4, S=2048, H=768, 12 heads x d=64) on 8 Trainium2
NeuronCores.

Sharding: core c handles batch b = c//2 and head group hg = c%2 (6 heads).
No cross-core communication; the host scatters inputs and gathers the output.

v2 vs baseline (350us):
  - inputs fed as bf16 (halves input DMA and LDWEIGHTS cost; PE rate for
    fp32r with N>=256 was already 1 cycle/row so matmul time is unchanged)
  - zero bias => contraction is exactly 6 chunks of 128 (768), no pad chunk
  - consecutive matmuls reuse the loaded stationary operand: the default
    backend flags carry --enable-ldw-opt=false; we flip it to true so
    walrus elides redundant LDWEIGHTS (the ISA caps one matmul at 512
    moving elements, so multi-bank streams must be separate instructions)
  - softmax exp as one 2048-elem ACTIVATE per j-tile (4 psum banks:
    2 heads x 2 i-blocks) cutting scalar-engine per-instruction overhead
  - reciprocal -> reciprocal_approx_fast (~5x faster on DVE)
  - chunk = (head pair, i-half of 1024); per j-tile of 128 tokens:
    scores (2 row-group-concurrent compound matmuls) -> exp -> ctx of the
    previous chunk (compound over the 2 i-blocks per v load)

PSUM budget (8 banks): scores/proj ring "s" [128,4,512] x1 = 4 banks;
ctx/v/fill ring "c" [128,2,512] x2 = 4 banks.  Projection fills that need
psum are scheduled only where a ring is free: pair0+pair1-q in the startup
window, v during chunk0 (no ctx yet), pair1-k/pair2 at chunk boundaries
between a finalize and the next ctx accumulation.

Per-core layouts (SBUF [128 partitions x free]):
  xT   [128, 6, 2048] bf16   x[b].T
  wq/wk/wv [128, 6, 384] bf16 weight column-slices for this head group
  qT/kT [128, 3, 2048] bf16  per head-pair stacked d-dims (even head: p0-63,
                             odd: p64-127)
  v    [128, 16, 6, 96] bf16 token-major v; cols 64:96 of each head are ones
                             so the ctx matmul leaves 32 copies of sumexp in
                             psum rows 64:96 (free softmax denominator)
  ss psum [128, 2h, 2ic, 512] scores -> exp -> ex sbuf bf16
  pc psum [96, 2ic, 512] ctx^T rows 0:64, sumexp copies rows 64:96
"""
import os

import numpy as np

if not os.environ.get("KERNEL_TRACE"):
    os.environ.setdefault("BASS_NEVER_TRACE", "1")

import concourse.bass as bass
import concourse.mybir as mybir
import concourse.tile as tile
from concourse import bacc
from concourse.bass import ts
from concourse.bass_utils import run_bass_kernel_spmd

import ml_dtypes

F32 = mybir.dt.float32
BF16 = mybir.dt.bfloat16

HIDDEN = 768
N_HEADS = 12
HEAD_DIM = 64
B = 4
S = 2048
HPC = 6          # heads per core
NPAIR = HPC // 2
NJ = S // 128    # 16 j-tiles of 128 tokens
VW = 96          # v (64) | ones (32)

_cache = {}
last_results = None


def _build(use_mask: bool, use_bias: bool):
    KC = 7 if use_bias else 6   # contraction chunks of 128
    nc = bacc.Bacc("TRN2", target_bir_lowering=False, debug=False, num_devices=8)

    xT_d = nc.dram_tensor("xT", [KC * 128, S], BF16, kind="ExternalInput")
    wq_d = nc.dram_tensor("wq", [KC * 128, HPC * HEAD_DIM], BF16, kind="ExternalInput")
    wk_d = nc.dram_tensor("wk", [KC * 128, HPC * HEAD_DIM], BF16, kind="ExternalInput")
    wv_d = nc.dram_tensor("wv", [KC * 128, HPC * HEAD_DIM], BF16, kind="ExternalInput")
    if use_mask:
        em_d = nc.dram_tensor("em", [128, NJ], F32, kind="ExternalInput")
    out_d = nc.dram_tensor("out", [HPC, HEAD_DIM, S], F32, kind="ExternalOutput")

    with tile.TileContext(nc) as tc:
        with (
            tc.tile_pool(name="const", bufs=1) as cpool,
            tc.tile_pool(name="qk", bufs=1) as qkpool,
            tc.tile_pool(name="vp", bufs=1) as vpool,
            tc.tile_pool(name="op", bufs=3) as opool,
            tc.tile_pool(name="rp", bufs=2) as rpool,
            tc.tile_pool(name="xw", bufs=1) as xwpool,
            tc.tile_pool(name="ex", bufs=6) as expool,
            tc.tile_pool(name="pss", bufs=1, space="PSUM") as pss,
            tc.tile_pool(name="psc", bufs=2, space="PSUM") as psc,
        ):
            if use_mask:
                em = cpool.tile([128, NJ], F32)
                nc.sync.dma_start(em[:], em_d[:])

            qT = qkpool.tile([128, NPAIR, S], BF16)
            kT = qkpool.tile([128, NPAIR, S], BF16)
            v = vpool.tile([128, NJ, HPC, VW], BF16)
            nc.vector.memset(v[:, :, :, HEAD_DIM:VW], 1.0)

            xT = xwpool.tile([128, KC, S], BF16)
            wq = xwpool.tile([128, KC, HPC * HEAD_DIM], BF16)
            wk = xwpool.tile([128, KC, HPC * HEAD_DIM], BF16)
            wv = xwpool.tile([128, KC, HPC * HEAD_DIM], BF16)
            # weights first (small), then x chunk-by-chunk so the first
            # projection matmuls start as soon as each chunk lands
            for c in range(KC):
                nc.sync.dma_start(wq[:, c, :], wq_d[ts(c, 128), :])
                nc.sync.dma_start(wk[:, c, :], wk_d[ts(c, 128), :])
            for c in range(KC):
                nc.sync.dma_start(xT[:, c, :], xT_d[ts(c, 128), :])
            for c in range(KC):
                nc.sync.dma_start(wv[:, c, :], wv_d[ts(c, 128), :])

            def emit_qk_pss(p, which):
                # one 4-bank psum group: per c, 4 N=512 matmuls sharing one
                # loaded stationary (ldw-opt elides the repeat LDWEIGHTS)
                w_, dst = (wq, qT) if which == 0 else (wk, kT)
                acc = pss.tile([128, 4, 512], F32, tag="s", name=f"qk{p}{which}")
                for c in range(KC):
                    for n in range(4):
                        nc.tensor.matmul(
                            acc[:, n, :], w_[:, c, ts(p, 128)],
                            xT[:, c, ts(n, 512)],
                            start=(c == 0), stop=(c == KC - 1),
                        )
                nc.vector.tensor_copy(
                    dst[:, p, :], acc[:].rearrange("p a n -> p (a n)")
                )

            def emit_qk_psc(p, which):
                # same projection as two 2-bank halves on the "c" ring
                w_, dst = (wq, qT) if which == 0 else (wk, kT)
                for half in range(2):
                    acc = psc.tile([128, 2, 512], F32, tag="c",
                                   name=f"qkh{p}{which}{half}")
                    for c in range(KC):
                        for n in range(2):
                            nc.tensor.matmul(
                                acc[:, n, :], w_[:, c, ts(p, 128)],
                                xT[:, c, ts(2 * half + n, 512)],
                                start=(c == 0), stop=(c == KC - 1),
                            )
                    nc.vector.tensor_copy(
                        dst[:, p, ts(half, 1024)],
                        acc[:].rearrange("p a n -> p (a n)"),
                    )

            def emit_qk_first():
                # pair0: q on the 4-bank "s" tile, k as two 2-bank "c" tiles,
                # interleaved per contraction chunk so everything streams
                # while the input DMA is still in flight
                qacc = pss.tile([128, 4, 512], F32, tag="s", name="q0acc")
                kacc0 = psc.tile([128, 2, 512], F32, tag="c", name="k0acc0")
                kacc1 = psc.tile([128, 2, 512], F32, tag="c", name="k0acc1")
                for c in range(KC):
                    for n in range(4):
                        nc.tensor.matmul(
                            qacc[:, n, :], wq[:, c, 0:128], xT[:, c, ts(n, 512)],
                            start=(c == 0), stop=(c == KC - 1),
                        )
                    for n in range(4):
                        ka = kacc0 if n < 2 else kacc1
                        nc.tensor.matmul(
                            ka[:, n % 2, :], wk[:, c, 0:128], xT[:, c, ts(n, 512)],
                            start=(c == 0), stop=(c == KC - 1),
                        )
                nc.vector.tensor_copy(
                    qT[:, 0, :], qacc[:].rearrange("p a n -> p (a n)")
                )
                nc.vector.tensor_copy(
                    kT[:, 0, 0:1024], kacc0[:].rearrange("p a n -> p (a n)")
                )
                nc.vector.tensor_copy(
                    kT[:, 0, 1024:2048], kacc1[:].rearrange("p a n -> p (a n)")
                )

            def emit_v(jt):
                # v projection for one j-tile: psum [128 tokens, 384]
                pv = psc.tile([128, 2, 512], F32, tag="c", name=f"pv{jt}")
                pvf = pv[:].rearrange("p a n -> p (a n)")[:, 0:HPC * HEAD_DIM]
                for c in range(KC):
                    nc.tensor.matmul(
                        pvf, xT[:, c, ts(jt, 128)], wv[:, c, :],
                        start=(c == 0), stop=(c == KC - 1),
                    )
                nc.vector.tensor_copy(
                    v[:, jt, :, 0:HEAD_DIM],
                    pvf.rearrange("p (h e) -> p h e", h=HPC),
                )

            # ex tiles hold 4 j-tiles each: [128, 4jt, 2head, 2ic, 512]
            def emit_scores_exp(pr_, ic2, jt, ex):
                ss = pss.tile([128, 2, 2, 512], F32, tag="s", name=f"ss{jt}")
                for a_ in range(2):
                    po = 64 * a_
                    for n in range(2):
                        nc.tensor.matmul(
                            ss[:, a_, n, :],
                            kT[po:po + 64, pr_, ts(jt, 128)],
                            qT[po:po + 64, pr_, ts(2 * ic2 + n, 512)],
                            start=True, stop=True,
                        )
                nc.scalar.activation(
                    ex[:, jt % 4, :, :, :], ss[:],
                    mybir.ActivationFunctionType.Exp,
                    scale=1.0 / np.sqrt(HEAD_DIM),
                )
                if use_mask:
                    nc.vector.tensor_scalar_mul(
                        ex[:, jt % 4, :, :, :], ex[:, jt % 4, :, :, :],
                        em[:, jt:jt + 1],
                    )

            def emit_ctx(pr_, pcs, jt, exs):
                ex = exs[jt // 4]
                for a_ in range(2):
                    for n in range(2):
                        nc.tensor.matmul(
                            pcs[a_][0:VW, n, :], v[:, jt, 2 * pr_ + a_, :],
                            ex[:, jt % 4, a_, n, :],
                            start=(jt == 0), stop=(jt == NJ - 1),
                        )

            def emit_finalize(pr_, ic2, pcs):
                for a_ in range(2):
                    h = 2 * pr_ + a_
                    pc = pcs[a_]
                    rc = rpool.tile([32, 2, 512], F32, tag="rc")
                    if os.environ.get("KERNEL_FAST_RECIP"):
                        nc.vector.reciprocal_approx_fast(rc[:], pc[64:VW, :, :])
                    else:
                        nc.vector.reciprocal(rc[:], pc[64:VW, :, :])
                    o = opool.tile([64, 2, 512], F32, tag="o")
                    nc.vector.tensor_tensor(
                        o[0:32, :, :], pc[0:32, :, :], rc[:],
                        op=mybir.AluOpType.mult,
                    )
                    nc.vector.tensor_tensor(
                        o[32:64, :, :], pc[32:64, :, :], rc[:],
                        op=mybir.AluOpType.mult,
                    )
                    nc.sync.dma_start(
                        out_d[h, :, ts(ic2, 1024)],
                        o[:].rearrange("p a n -> p (a n)"),
                    )

            # ---- schedule ----
            # startup: pair0 q/k (DMA-gated window), then pair1 q on the
            # "s" ring before the first scores tile
            emit_qk_first()
            emit_qk_pss(1, 0)

            # psc-ring fills placed at chunk boundaries (after the previous
            # finalize, before the next ctx accumulators claim the ring)
            boundary_fills = {
                1: [lambda: emit_qk_psc(1, 1)],                  # k pair1
                2: [lambda: emit_qk_psc(2, 0)],                  # q pair2
                3: [lambda: emit_qk_psc(2, 1)],                  # k pair2
            }

            prev = None  # (pr, ic2, pcs, exs) of previous chunk
            for CH in range(2 * NPAIR):
                pr_, ic2 = CH // 2, CH % 2
                for fill in boundary_fills.get(CH, []):
                    fill()
                pcs = None
                if prev is not None:
                    pcs = [psc.tile([128, 2, 512], F32, tag="c",
                                    name=f"pc{CH}_{a}") for a in range(2)]
                exs = []
                for jt in range(NJ):
                    if jt % 4 == 0:
                        ex = expool.tile([128, 4, 2, 2, 512], BF16, tag="e")
                        exs.append(ex)
                    emit_scores_exp(pr_, ic2, jt, exs[jt // 4])
                    if CH == 0:
                        emit_v(jt)
                    if prev is not None:
                        emit_ctx(prev[0], pcs, jt, prev[3])
                if prev is not None:
                    emit_finalize(prev[0], prev[1], pcs)
                prev = (pr_, ic2, pcs, exs)

            # last chunk's ctx + finalize (trails the act stream closely)
            pcs = [psc.tile([128, 2, 512], F32, tag="c", name=f"pcL_{a}")
                   for a in range(2)]
            for jt in range(NJ):
                emit_ctx(prev[0], pcs, jt, prev[3])
            emit_finalize(prev[0], prev[1], pcs)

    nc.compile()
    return nc


def _enable_ldw_opt():
    # The default backend options carry --enable-ldw-opt=false, which makes
    # walrus emit one LDWEIGHTS per matmul even when consecutive matmuls
    # share the stationary operand. Flip it for this process's compiles.
    from concourse import compiler_utils

    flags = compiler_utils.get_compiler_flags()
    patched = [f.replace("--enable-ldw-opt=false", "--enable-ldw-opt=true")
               for f in flags]
    if patched != flags:
        compiler_utils.set_compiler_flags(patched)


def _get_nc(use_mask: bool, use_bias: bool):
    key = (use_mask, use_bias)
    if key not in _cache:
        if os.environ.get("KERNEL_LDW_OPT"):
            _enable_ldw_opt()
        _cache[key] = _build(use_mask, use_bias)
    return _cache[key]


def kernel(hidden_states, attention_mask, Wq, bq, Wk, bk, Wv, bv):
    global last_results
    hidden_states = np.asarray(hidden_states, dtype=np.float32)
    attention_mask = np.asarray(attention_mask, dtype=np.float32)
    Wq = np.asarray(Wq, dtype=np.float32)
    Wk = np.asarray(Wk, dtype=np.float32)
    Wv = np.asarray(Wv, dtype=np.float32)
    bq = np.asarray(bq, dtype=np.float32)
    bk = np.asarray(bk, dtype=np.float32)
    bv = np.asarray(bv, dtype=np.float32)

    use_mask = bool(np.any(attention_mask))
    use_bias = bool(np.any(bq) or np.any(bk) or np.any(bv))
    nc = _get_nc(use_mask, use_bias)
    KC = 7 if use_bias else 6
    bf16 = ml_dtypes.bfloat16

    in_maps = []
    for c in range(8):
        b = c // 2
        hg = c % 2
        cs = slice(hg * HPC * HEAD_DIM, (hg + 1) * HPC * HEAD_DIM)

        xT = np.zeros((KC * 128, S), dtype=np.float32)
        xT[:HIDDEN] = hidden_states[b].T
        if use_bias:
            xT[HIDDEN] = 1.0

        def wslice(W, bias):
            w = np.zeros((KC * 128, HPC * HEAD_DIM), dtype=np.float32)
            w[:HIDDEN] = W[:, cs]
            if use_bias:
                w[HIDDEN] = bias[cs]
            return w

        m = {
            "xT": xT.astype(bf16),
            "wq": wslice(Wq, bq).astype(bf16),
            "wk": wslice(Wk, bk).astype(bf16),
            "wv": wslice(Wv, bv).astype(bf16),
        }
        if use_mask:
            em = np.exp(attention_mask[b, 0, 0, :]).astype(np.float32)
            m["em"] = np.ascontiguousarray(em.reshape(NJ, 128).T)
        in_maps.append(m)

    res = run_bass_kernel_spmd(
        nc, in_maps, list(range(8)),
        trace=bool(os.environ.get("KERNEL_TRACE")),
    )
    last_results = res

    out = np.empty((B, S, HIDDEN), dtype=np.float32)
    for c in range(8):
        b = c // 2
        hg = c % 2
        r = res.results[c]["out"]  # [6, 64, 2048]
        out[b, :, hg * HPC * HEAD_DIM:(hg + 1) * HPC * HEAD_DIM] = (
            r.transpose(2, 0, 1).reshape(S, HPC * HEAD_DIM)
        )
    return out


# revision 17
# speedup vs baseline: 1.0538x; 1.0538x over previous
"""BERT self-attention (B=4, S=2048, H=768, 12 heads x d=64) on 8 Trainium2
NeuronCores.

Sharding: core c handles batch b = c//2 and head group hg = c%2 (6 heads).
No cross-core communication; the host scatters inputs and gathers the output.

v2 vs baseline (350us):
  - inputs fed as bf16 (halves input DMA and LDWEIGHTS cost; PE rate for
    fp32r with N>=256 was already 1 cycle/row so matmul time is unchanged)
  - zero bias => contraction is exactly 6 chunks of 128 (768), no pad chunk
  - consecutive matmuls reuse the loaded stationary operand: the default
    backend flags carry --enable-ldw-opt=false; we flip it to true so
    walrus elides redundant LDWEIGHTS (the ISA caps one matmul at 512
    moving elements, so multi-bank streams must be separate instructions)
  - softmax exp as one 2048-elem ACTIVATE per j-tile (4 psum banks:
    2 heads x 2 i-blocks) cutting scalar-engine per-instruction overhead
  - reciprocal -> reciprocal_approx_fast (~5x faster on DVE)
  - chunk = (head pair, i-half of 1024); per j-tile of 128 tokens:
    scores (2 row-group-concurrent compound matmuls) -> exp -> ctx of the
    previous chunk (compound over the 2 i-blocks per v load)

PSUM budget (8 banks): scores/proj ring "s" [128,4,512] x1 = 4 banks;
ctx/v/fill ring "c" [128,2,512] x2 = 4 banks.  Projection fills that need
psum are scheduled only where a ring is free: pair0+pair1-q in the startup
window, v during chunk0 (no ctx yet), pair1-k/pair2 at chunk boundaries
between a finalize and the next ctx accumulation.

Per-core layouts (SBUF [128 partitions x free]):
  xT   [128, 6, 2048] bf16   x[b].T
  wq/wk/wv [128, 6, 384] bf16 weight column-slices for this head group
  qT/kT [128, 3, 2048] bf16  per head-pair stacked d-dims (even head: p0-63,
                             odd: p64-127)
  v    [128, 16, 6, 96] bf16 token-major v; cols 64:96 of each head are ones
                             so the ctx matmul leaves 32 copies of sumexp in
                             psum rows 64:96 (free softmax denominator)
  ss psum [128, 2h, 2ic, 512] scores -> exp -> ex sbuf bf16
  pc psum [96, 2ic, 512] ctx^T rows 0:64, sumexp copies rows 64:96
"""
import os

import numpy as np

if not os.environ.get("KERNEL_TRACE"):
    os.environ.setdefault("BASS_NEVER_TRACE", "1")

import concourse.bass as bass
import concourse.mybir as mybir
import concourse.tile as tile
from concourse import bacc
from concourse.bass import ts
from concourse.bass_utils import run_bass_kernel_spmd

import ml_dtypes

F32 = mybir.dt.float32
BF16 = mybir.dt.bfloat16

HIDDEN = 768
N_HEADS = 12
HEAD_DIM = 64
B = 4
S = 2048
HPC = 6          # heads per core
NPAIR = HPC // 2
NJ = S // 128    # 16 j-tiles of 128 tokens
VW = 96          # v (64) | ones (32)

_cache = {}
last_results = None


def _build(use_mask: bool, use_bias: bool):
    KC = 7 if use_bias else 6   # contraction chunks of 128
    nc = bacc.Bacc("TRN2", target_bir_lowering=False, debug=False, num_devices=8)

    xT_d = nc.dram_tensor("xT", [KC * 128, S], BF16, kind="ExternalInput")
    wq_d = nc.dram_tensor("wq", [KC * 128, HPC * HEAD_DIM], BF16, kind="ExternalInput")
    wk_d = nc.dram_tensor("wk", [KC * 128, HPC * HEAD_DIM], BF16, kind="ExternalInput")
    wv_d = nc.dram_tensor("wv", [KC * 128, HPC * HEAD_DIM], BF16, kind="ExternalInput")
    if use_mask:
        em_d = nc.dram_tensor("em", [128, NJ], F32, kind="ExternalInput")
    out_d = nc.dram_tensor("out", [HPC, HEAD_DIM, S], F32, kind="ExternalOutput")

    with tile.TileContext(nc) as tc:
        with (
            tc.tile_pool(name="const", bufs=1) as cpool,
            tc.tile_pool(name="qk", bufs=1) as qkpool,
            tc.tile_pool(name="vp", bufs=1) as vpool,
            tc.tile_pool(name="op", bufs=1) as opool,
            tc.tile_pool(name="rp", bufs=1) as rpool,
            tc.tile_pool(name="st", bufs=2) as stpool,
            tc.tile_pool(name="xw", bufs=1) as xwpool,
            tc.tile_pool(name="ex", bufs=6) as expool,
            tc.tile_pool(name="pss", bufs=1, space="PSUM") as pss,
            tc.tile_pool(name="psc", bufs=2, space="PSUM") as psc,
        ):
            if use_mask:
                em = cpool.tile([128, NJ], F32)
                nc.sync.dma_start(em[:], em_d[:])

            qT = qkpool.tile([128, NPAIR, S], BF16)
            kT = qkpool.tile([128, NPAIR, S], BF16)
            v = vpool.tile([128, NJ, HPC, VW], BF16)
            nc.vector.memset(v[:, :, :, HEAD_DIM:VW], 1.0)

            xT = xwpool.tile([128, KC, S], BF16)
            wq = xwpool.tile([128, KC, HPC * HEAD_DIM], BF16)
            wk = xwpool.tile([128, KC, HPC * HEAD_DIM], BF16)
            wv = xwpool.tile([128, KC, HPC * HEAD_DIM], BF16)
            # weights first (small), then x chunk-by-chunk so the first
            # projection matmuls start as soon as each chunk lands
            for c in range(KC):
                nc.sync.dma_start(wq[:, c, :], wq_d[ts(c, 128), :])
                nc.sync.dma_start(wk[:, c, :], wk_d[ts(c, 128), :])
            for c in range(KC):
                nc.sync.dma_start(xT[:, c, :], xT_d[ts(c, 128), :])
            for c in range(KC):
                nc.sync.dma_start(wv[:, c, :], wv_d[ts(c, 128), :])

            def emit_qk_pss(p, which):
                # one 4-bank psum group: per c, 4 N=512 matmuls sharing one
                # loaded stationary (ldw-opt elides the repeat LDWEIGHTS)
                w_, dst = (wq, qT) if which == 0 else (wk, kT)
                acc = pss.tile([128, 4, 512], F32, tag="s", name=f"qk{p}{which}")
                for c in range(KC):
                    for n in range(4):
                        nc.tensor.matmul(
                            acc[:, n, :], w_[:, c, ts(p, 128)],
                            xT[:, c, ts(n, 512)],
                            start=(c == 0), stop=(c == KC - 1),
                        )
                nc.vector.tensor_copy(
                    dst[:, p, :], acc[:].rearrange("p a n -> p (a n)")
                )

            def emit_qk_psc(p, which):
                # same projection as two 2-bank halves on the "c" ring
                w_, dst = (wq, qT) if which == 0 else (wk, kT)
                for half in range(2):
                    acc = psc.tile([128, 2, 512], F32, tag="c",
                                   name=f"qkh{p}{which}{half}")
                    for c in range(KC):
                        for n in range(2):
                            nc.tensor.matmul(
                                acc[:, n, :], w_[:, c, ts(p, 128)],
                                xT[:, c, ts(2 * half + n, 512)],
                                start=(c == 0), stop=(c == KC - 1),
                            )
                    nc.vector.tensor_copy(
                        dst[:, p, ts(half, 1024)],
                        acc[:].rearrange("p a n -> p (a n)"),
                    )

            def emit_qk_first():
                # pair0: q on the 4-bank "s" tile, k as two 2-bank "c" tiles,
                # interleaved per contraction chunk so everything streams
                # while the input DMA is still in flight
                qacc = pss.tile([128, 4, 512], F32, tag="s", name="q0acc")
                kacc0 = psc.tile([128, 2, 512], F32, tag="c", name="k0acc0")
                kacc1 = psc.tile([128, 2, 512], F32, tag="c", name="k0acc1")
                for c in range(KC):
                    for n in range(4):
                        nc.tensor.matmul(
                            qacc[:, n, :], wq[:, c, 0:128], xT[:, c, ts(n, 512)],
                            start=(c == 0), stop=(c == KC - 1),
                        )
                    for n in range(4):
                        ka = kacc0 if n < 2 else kacc1
                        nc.tensor.matmul(
                            ka[:, n % 2, :], wk[:, c, 0:128], xT[:, c, ts(n, 512)],
                            start=(c == 0), stop=(c == KC - 1),
                        )
                nc.vector.tensor_copy(
                    qT[:, 0, :], qacc[:].rearrange("p a n -> p (a n)")
                )
                nc.vector.tensor_copy(
                    kT[:, 0, 0:1024], kacc0[:].rearrange("p a n -> p (a n)")
                )
                nc.vector.tensor_copy(
                    kT[:, 0, 1024:2048], kacc1[:].rearrange("p a n -> p (a n)")
                )

            def emit_v(jt):
                # v projection for one j-tile: psum [128 tokens, 384]
                pv = psc.tile([128, 2, 512], F32, tag="c", name=f"pv{jt}")
                pvf = pv[:].rearrange("p a n -> p (a n)")[:, 0:HPC * HEAD_DIM]
                for c in range(KC):
                    nc.tensor.matmul(
                        pvf, xT[:, c, ts(jt, 128)], wv[:, c, :],
                        start=(c == 0), stop=(c == KC - 1),
                    )
                nc.vector.tensor_copy(
                    v[:, jt, :, 0:HEAD_DIM],
                    pvf.rearrange("p (h e) -> p h e", h=HPC),
                )

            # ex tiles hold 4 j-tiles each: [128, 4jt, 2head, 2ic, 512]
            def emit_scores_exp(pr_, ic2, jt, ex):
                ss = pss.tile([128, 2, 2, 512], F32, tag="s", name=f"ss{jt}")
                for a_ in range(2):
                    po = 64 * a_
                    for n in range(2):
                        nc.tensor.matmul(
                            ss[:, a_, n, :],
                            kT[po:po + 64, pr_, ts(jt, 128)],
                            qT[po:po + 64, pr_, ts(2 * ic2 + n, 512)],
                            start=True, stop=True,
                        )
                nc.scalar.activation(
                    ex[:, jt % 4, :, :, :], ss[:],
                    mybir.ActivationFunctionType.Exp,
                    scale=1.0 / np.sqrt(HEAD_DIM),
                )
                if use_mask:
                    nc.vector.tensor_scalar_mul(
                        ex[:, jt % 4, :, :, :], ex[:, jt % 4, :, :, :],
                        em[:, jt:jt + 1],
                    )

            def emit_ctx(pr_, pcs, jt, exs):
                ex = exs[jt // 4]
                for a_ in range(2):
                    for n in range(2):
                        nc.tensor.matmul(
                            pcs[a_][0:VW, n, :], v[:, jt, 2 * pr_ + a_, :],
                            ex[:, jt % 4, a_, n, :],
                            start=(jt == 0), stop=(jt == NJ - 1),
                        )

            def emit_finalize(pr_, ic2, pcs):
                # Free the "c" psum ring fast: stage ctx rows (bf16) and the
                # sumexp rows (f32) to SBUF with three quick DVE copies per
                # head (~2.3us), then normalize off-ring: DVE reciprocal of
                # the staged sumexp, multiplies on the idle gpsimd engine,
                # DMA out. All SBUF-SBUF operands start at partition 0 (the
                # walrus verifier requires aligned SB start partitions; psum
                # sources are exempt).
                stages = []
                for a_ in range(2):
                    sl = stpool.tile([32, 2, 512], BF16, tag="sl")
                    sh = stpool.tile([32, 2, 512], BF16, tag="sh")
                    se = stpool.tile([32, 2, 512], F32, tag="se")
                    nc.vector.tensor_copy(sl[:], pcs[a_][0:32, :, :])
                    nc.vector.tensor_copy(sh[:], pcs[a_][32:64, :, :])
                    nc.vector.tensor_copy(se[:], pcs[a_][64:VW, :, :])
                    stages.append((sl, sh, se))
                for a_ in range(2):
                    h = 2 * pr_ + a_
                    sl, sh, se = stages[a_]
                    rc = rpool.tile([32, 2, 512], F32, tag="rc")
                    nc.vector.reciprocal(rc[:], se[:])
                    o = opool.tile([32, 2, 2, 512], F32, tag="o")
                    nc.gpsimd.tensor_tensor(
                        o[:, :, 0, :], sl[:], rc[:], op=mybir.AluOpType.mult
                    )
                    nc.gpsimd.tensor_tensor(
                        o[:, :, 1, :], sh[:], rc[:], op=mybir.AluOpType.mult
                    )
                    nc.sync.dma_start(
                        out_d[h, 0:32, ts(ic2, 1024)], o[:, :, 0, :]
                    )
                    nc.sync.dma_start(
                        out_d[h, 32:64, ts(ic2, 1024)], o[:, :, 1, :]
                    )

            # ---- schedule ----
            # warm the PE p-state during the input DMA window: ~24 tiny
            # matmuls on the already-memset ones region keep the tensor
            # engine continuously busy so the real projections start at
            # full clock instead of the 2x-slow mid p-state
            warm = pss.tile([128, 4, 512], F32, tag="s", name="warm")
            ones32 = v[:, 0, 0, HEAD_DIM:VW]
            for _ in range(24):
                nc.tensor.matmul(warm[0:32, 0, 0:32], ones32, ones32,
                                 start=True, stop=True)

            # startup: pair0 q/k (DMA-gated window), then pair1 q on the
            # "s" ring before the first scores tile
            emit_qk_first()
            emit_qk_pss(1, 0)

            # psc-ring fills placed at chunk boundaries (after the previous
            # finalize, before the next ctx accumulators claim the ring)
            boundary_fills = {
                1: [lambda: emit_qk_psc(1, 1)],                  # k pair1
                2: [lambda: emit_qk_psc(2, 0)],                  # q pair2
                3: [lambda: emit_qk_psc(2, 1)],                  # k pair2
            }

            prev = None  # (pr, ic2, pcs, exs) of previous chunk
            for CH in range(2 * NPAIR):
                pr_, ic2 = CH // 2, CH % 2
                for fill in boundary_fills.get(CH, []):
                    fill()
                pcs = None
                if prev is not None:
                    pcs = [psc.tile([128, 2, 512], F32, tag="c",
                                    name=f"pc{CH}_{a}") for a in range(2)]
                exs = []
                for jt in range(NJ):
                    if jt % 4 == 0:
                        ex = expool.tile([128, 4, 2, 2, 512], BF16, tag="e")
                        exs.append(ex)
                    emit_scores_exp(pr_, ic2, jt, exs[jt // 4])
                    if CH == 0:
                        emit_v(jt)
                    if prev is not None:
                        emit_ctx(prev[0], pcs, jt, prev[3])
                if prev is not None:
                    emit_finalize(prev[0], prev[1], pcs)
                prev = (pr_, ic2, pcs, exs)

            # last chunk's ctx + finalize (trails the act stream closely)
            pcs = [psc.tile([128, 2, 512], F32, tag="c", name=f"pcL_{a}")
                   for a in range(2)]
            for jt in range(NJ):
                emit_ctx(prev[0], pcs, jt, prev[3])
            emit_finalize(prev[0], prev[1], pcs)

    nc.compile()
    return nc


def _enable_ldw_opt():
    # The default backend options carry --enable-ldw-opt=false, which makes
    # walrus emit one LDWEIGHTS per matmul even when consecutive matmuls
    # share the stationary operand. Flip it for this process's compiles.
    from concourse import compiler_utils

    flags = compiler_utils.get_compiler_flags()
    patched = [f.replace("--enable-ldw-opt=false", "--enable-ldw-opt=true")
               for f in flags]
    if patched != flags:
        compiler_utils.set_compiler_flags(patched)


def _get_nc(use_mask: bool, use_bias: bool):
    key = (use_mask, use_bias)
    if key not in _cache:
        if os.environ.get("KERNEL_LDW_OPT"):
            _enable_ldw_opt()
        _cache[key] = _build(use_mask, use_bias)
    return _cache[key]


def kernel(hidden_states, attention_mask, Wq, bq, Wk, bk, Wv, bv):
    global last_results
    hidden_states = np.asarray(hidden_states, dtype=np.float32)
    attention_mask = np.asarray(attention_mask, dtype=np.float32)
    Wq = np.asarray(Wq, dtype=np.float32)
    Wk = np.asarray(Wk, dtype=np.float32)
    Wv = np.asarray(Wv, dtype=np.float32)
    bq = np.asarray(bq, dtype=np.float32)
    bk = np.asarray(bk, dtype=np.float32)
    bv = np.asarray(bv, dtype=np.float32)

    use_mask = bool(np.any(attention_mask))
    use_bias = bool(np.any(bq) or np.any(bk) or np.any(bv))
    nc = _get_nc(use_mask, use_bias)
    KC = 7 if use_bias else 6
    bf16 = ml_dtypes.bfloat16

    in_maps = []
    for c in range(8):
        b = c // 2
        hg = c % 2
        cs = slice(hg * HPC * HEAD_DIM, (hg + 1) * HPC * HEAD_DIM)

        xT = np.zeros((KC * 128, S), dtype=np.float32)
        xT[:HIDDEN] = hidden_states[b].T
        if use_bias:
            xT[HIDDEN] = 1.0

        def wslice(W, bias):
            w = np.zeros((KC * 128, HPC * HEAD_DIM), dtype=np.float32)
            w[:HIDDEN] = W[:, cs]
            if use_bias:
                w[HIDDEN] = bias[cs]
            return w

        m = {
            "xT": xT.astype(bf16),
            "wq": wslice(Wq, bq).astype(bf16),
            "wk": wslice(Wk, bk).astype(bf16),
            "wv": wslice(Wv, bv).astype(bf16),
        }
        if use_mask:
            em = np.exp(attention_mask[b, 0, 0, :]).astype(np.float32)
            m["em"] = np.ascontiguousarray(em.reshape(NJ, 128).T)
        in_maps.append(m)

    res = run_bass_kernel_spmd(
        nc, in_maps, list(range(8)),
        trace=bool(os.environ.get("KERNEL_TRACE")),
    )
    last_results = res

    out = np.empty((B, S, HIDDEN), dtype=np.float32)
    for c in range(8):
        b = c // 2
        hg = c % 2
        r = res.results[c]["out"]  # [6, 64, 2048]
        out[b, :, hg * HPC * HEAD_DIM:(hg + 1) * HPC * HEAD_DIM] = (
            r.transpose(2, 0, 1).reshape(S, HPC * HEAD_DIM)
        )
    return out
